# revision 8
# baseline (speedup 1.0000x reference)
"""MoE routing kernel for Trainium2 (8 NeuronCores, expert-parallel).

Strategy (per spec sharding_hint):
  - Host computes the tiny gating Dense + softmax + top-2 routing (0.02% of
    the FLOPs) in float64 -- this decides the sharding, so it must run on
    host before dispatch.
  - Tokens are dispatched to expert-owning cores: core e receives the tokens
    whose top-2 includes expert e, pre-scaled by their combine weight and
    laid out transposed/tiled so the device DMA is fully contiguous.
  - Each core runs one dense [cap x 2048] @ [2048 x 2048] matmul against its
    resident expert weight (float32r on the PE at full rate, fp32 accuracy
    class), streaming token tiles, with weights loaded to SBUF exactly once.
  - Host scatters per-expert outputs back (y[idx] += Y_e) and adds the
    combine-weighted bias term.
"""

import numpy as np

N_TOKENS = 8192
D_IN = 2048
HIDDEN = 2048
NUM_EXPERTS = 8
TOP_K = 2
P = 128
NFREE = 512  # matmul moving free dim (one PSUM bank of fp32)

_KERNEL_CACHE: dict[int, object] = {}
LAST_EXEC_NS = None
LAST_TRACE = None


def _build_bass_kernel(cap: int):
    """Build + schedule the per-core Bass program for capacity `cap` tokens."""
    import concourse.bacc as bacc
    import concourse.tile as tile
    import concourse.mybir as mybir

    KO = D_IN // P        # 16 contraction tiles
    MT = cap // P         # token tiles
    NT = HIDDEN // NFREE  # 4 output column chunks

    nc = bacc.Bacc("TRN2", target_bir_lowering=False, debug=False)

    # xT layout: [ki, m_tile, ko, mi]  (value = xg[m*128+mi, ko*128+ki])
    xT = nc.dram_tensor("xT", [P, MT, KO, P], mybir.dt.float32r, kind="ExternalInput")
    w = nc.dram_tensor("w", [D_IN, HIDDEN], mybir.dt.float32r, kind="ExternalInput")
    y = nc.dram_tensor("y", [cap, HIDDEN], mybir.dt.float32, kind="ExternalOutput")

    with tile.TileContext(nc) as tc:
        with (
            tc.tile_pool(name="wpool", bufs=1) as wpool,
            tc.tile_pool(name="xpool", bufs=3) as xpool,
            tc.tile_pool(name="opool", bufs=6) as opool,
            tc.tile_pool(name="psum", bufs=2, space="PSUM") as psum_pool,
        ):
            # Expert weight resident in SBUF: one tile per ko so matmuls can
            # start as soon as their slice lands.
            w_k = []
            for ko in range(KO):
                wt = wpool.tile([P, HIDDEN], mybir.dt.float32r, tag=f"w{ko}")
                nc.sync.dma_start(out=wt[:], in_=w[ko * P:(ko + 1) * P, :])
                w_k.append(wt)

            for m in range(MT):
                xm = xpool.tile([P, KO, P], mybir.dt.float32r, tag="xm")
                nc.sync.dma_start(out=xm[:], in_=xT[:, m, :, :])
                # ko outer / n inner: the stationary operand (xm[:, ko, :])
                # stays loaded across the NT consecutive matmuls.
                ps = [
                    psum_pool.tile([P, NFREE], mybir.dt.float32, tag=f"ps{n}", name=f"ps_{m}_{n}")
                    for n in range(NT)
                ]
                for ko in range(KO):
                    for n in range(NT):
                        nc.tensor.matmul(
                            ps[n][:],
                            lhsT=xm[:, ko, :],
                            rhs=w_k[ko][:, n * NFREE:(n + 1) * NFREE],
                            start=(ko == 0),
                            stop=(ko == KO - 1),
                        )
                for n in range(NT):
                    ot = opool.tile([P, NFREE], mybir.dt.float32, tag="ot")
                    nc.any.tensor_copy(out=ot[:], in_=ps[n][:])
                    nc.sync.dma_start(
                        out=y[m * P:(m + 1) * P, n * NFREE:(n + 1) * NFREE],
                        in_=ot[:],
                    )

    nc.compile()
    return nc


def _route(x, Wg, bg):
    """Host gating in float64: softmax + top-2 (ties -> lower index, matching
    jax.lax.top_k)."""
    logits = x.astype(np.float64) @ Wg.astype(np.float64) + bg.astype(np.float64)
    logits -= logits.max(axis=-1, keepdims=True)
    p = np.exp(logits)
    p /= p.sum(axis=-1, keepdims=True)
    order = np.argsort(-p, axis=-1, kind="stable")
    top_idx = order[:, :TOP_K]                      # [N, K]
    top_w = np.take_along_axis(p, top_idx, axis=-1)  # [N, K]
    return top_idx, top_w.astype(np.float32)


def kernel(x, Wg, bg, W, b):
    x = np.asarray(x, dtype=np.float32)
    Wg = np.asarray(Wg, dtype=np.float32)
    bg = np.asarray(bg, dtype=np.float32)
    W = np.asarray(W, dtype=np.float32)
    b = np.asarray(b, dtype=np.float32)

    top_idx, top_w = _route(x, Wg, bg)

    # Per-expert token lists (an expert appears at most once per token).
    idx_e = []
    wgt_e = []
    for e in range(NUM_EXPERTS):
        hit = (top_idx == e)                        # [N, K] bool
        rows = np.nonzero(hit.any(axis=1))[0]
        wts = (top_w * hit).sum(axis=1)[rows].astype(np.float32)
        idx_e.append(rows)
        wgt_e.append(wts)

    counts = np.array([len(r) for r in idx_e])
    cap = max(P, int(-(-counts.max() // P)) * P)

    # The trimmed container lacks antenv.axon_hooks; stub it so a BASS_TRACE
    # request degrades to an untraced run instead of crashing.
    try:
        import antenv.axon_hooks  # noqa: F401
    except ImportError:
        import sys as _sys
        import types as _types

        _m = _types.ModuleType("antenv.axon_hooks")
        _m.get_axon_ntff_profile_hook = lambda: None
        _sys.modules["antenv.axon_hooks"] = _m

    from concourse import bass_utils

    nc = _KERNEL_CACHE.get(cap)
    if nc is None:
        nc = _build_bass_kernel(cap)
        _KERNEL_CACHE[cap] = nc

    KO = D_IN // P
    MT = cap // P

    in_maps = []
    for e in range(NUM_EXPERTS):
        xg = np.zeros((cap, D_IN), dtype=np.float32)
        xg[: counts[e]] = x[idx_e[e]] * wgt_e[e][:, None]
        # [cap, D] -> [ki, m_tile, ko, mi]
        xT = np.ascontiguousarray(
            xg.reshape(MT, P, KO, P).transpose(3, 0, 2, 1)
        )
        in_maps.append({"xT": xT, "w": np.ascontiguousarray(W[e])})

    import time as _time

    _t0 = _time.time()
    res = bass_utils.run_bass_kernel_spmd(
        nc, in_maps, core_ids=list(range(NUM_EXPERTS))
    )
    global LAST_EXEC_NS, LAST_TRACE, LAST_RUN_S
    LAST_RUN_S = _time.time() - _t0
    LAST_EXEC_NS = res.exec_time_ns
    LAST_TRACE = res.instructions_and_trace

    # Host combine: scatter-add expert outputs + combine-weighted bias.
    y = np.zeros((N_TOKENS, HIDDEN), dtype=np.float32)
    for e in range(NUM_EXPERTS):
        ye = res.results[e]["y"]
        y[idx_e[e]] += ye[: counts[e]]
        y[idx_e[e]] += wgt_e[e][:, None] * b[e][None, :]
    return y
